# revision 29
# baseline (speedup 1.0000x reference)
"""Trainium2 Bass kernel for the DCT-CNN expert core.

Reference computation (per 512x512 single-channel image):
  1. split into 4096 non-overlapping 8x8 patches
  2. 2D DCT per patch:  c = D @ p @ D^T
  3. conv3x3(1->16, SAME) + bias + relu on each 8x8 patch image
  4. conv3x3(16->32, SAME) + bias
  5. mean over spatial (8x8), then mean over patches  -> [B, 32]

Algebraic restructuring used here (validated to fp32 roundoff):
  - DCT + conv1 are both linear maps on the 64 patch pixels, so they fold
    into a single [1024, 64] matrix  W = M1 @ (D (x) D)  with bias b1
    broadcast per channel:    h1 = relu(W @ p + b1h)        [1024 per patch]
  - conv2 + spatial mean + patch mean are linear in h1, so they fold into
    a single [1024, 32] matrix applied to the per-image SUM of h1:
       out[b] = (sum_patches h1)^T @ M2e + b2
    where M2e includes the /64 spatial mean, /4096 patch mean.

Device work per core (2 images = 8192 patches):
  - 128 matmuls [K=64, M=128, N=512] (float32r) -> PSUM
  - 32 fused relu+bias+accumulate ops ([128, 2048], split ScalarE/VectorE)
  - tiny final reduction + [128,2]x[128,32] matmuls + bias add

Sharding: pure data parallel over images (2 per core), weights replicated.
"""
import numpy as np

import concourse.bass as bass
import concourse.bacc as bacc
import concourse.tile as tile
from concourse import mybir
from concourse.bass_utils import run_bass_kernel_spmd

N_CORES = 8
F32 = mybir.dt.float32
F32R = mybir.dt.float32r
BF16 = mybir.dt.bfloat16

try:
    import ml_dtypes
    NP_BF16 = np.dtype(ml_dtypes.bfloat16)
except ImportError:  # pragma: no cover
    NP_BF16 = None

# ---------------------------------------------------------------- host math

def _dct_matrix(n=8):
    m = np.zeros((n, n), dtype=np.float64)
    for k in range(n):
        for t in range(n):
            if k == 0:
                m[k, t] = 1.0 / np.sqrt(n)
            else:
                m[k, t] = np.sqrt(2.0 / n) * np.cos(np.pi * k * (2 * t + 1) / (2.0 * n))
    return m


def _conv3x3_matrix(w):
    """Dense linear operator of a SAME 3x3 cross-correlation on 8x8 images.

    w: [O, I, 3, 3] -> M: [O*64, I*64] with
    flatten(conv(img))[(o,y,x)] = sum M[(o,y,x),(i,r,c)] img[i,r,c]
    """
    O, I = w.shape[0], w.shape[1]
    M = np.zeros((O, 8, 8, I, 8, 8))
    for dy in range(3):
        for dx in range(3):
            ylo, yhi = max(0, 1 - dy), min(8, 9 - dy)
            xlo, xhi = max(0, 1 - dx), min(8, 9 - dx)
            for y in range(ylo, yhi):
                for x in range(xlo, xhi):
                    M[:, y, x, :, y + dy - 1, x + dx - 1] += w[:, :, dy, dx]
    return M.reshape(O * 64, I * 64)


def _build_weights(w1, b1, w2, b2):
    """Returns (Wt [64,1024], b1c [128,8], M2c [128,256], b2t [128,32]) f32."""
    D = _dct_matrix()
    KRON = np.kron(D, D)                                   # c_flat = KRON @ p_flat
    M1 = _conv3x3_matrix(w1.astype(np.float64))            # [1024, 64]
    M1K = M1 @ KRON                                        # [1024, 64]
    b1h = np.repeat(b1.astype(np.float64), 64)             # [1024]
    M2 = _conv3x3_matrix(w2.astype(np.float64))            # [2048, 1024]
    A2 = M2.reshape(32, 64, 1024).sum(axis=1)              # [32, 1024]
    M2e = A2.T / (64.0 * 4096.0)                           # [1024, 32]

    Wt = np.ascontiguousarray(M1K.T, dtype=np.float32)     # [64, 1024]
    b1c = np.ascontiguousarray(
        b1h.reshape(8, 128).T, dtype=np.float32)           # [128, 8]
    M2c = np.ascontiguousarray(
        M2e.reshape(8, 128, 32).transpose(1, 0, 2).reshape(128, 256),
        dtype=np.float32)                                  # [128, 8*32]
    b2t = np.ascontiguousarray(
        np.tile(b2.astype(np.float32), (128, 1)))          # [128, 32]
    return Wt, b1c, M2c, b2t


# ------------------------------------------------------------- device kernel

# wts dram param [128, 1024]: W duplicated on both partition halves
#   ([0:64) and [64:128)); consumed as float32r by the main matmuls.
# aux layout (f32 columns):
#   [0:8)      b1 chunks (col k = b1h[128k:128k+128])
#   [8:264)    M2e chunks (cols 32k..32k+32 = M2e[128k:128k+128, :])
#   [264:296)  b2 broadcast to all partitions
AUXB1 = 0
AUXM2 = 8
AUXB2 = 264
AUXTOT = 296

# relu engine assignment: ~17/32 groups on ScalarE (ACT), rest on VectorE.
_N_GROUPS = 32
_ACT_SHARE = 17


def _build_nc():
    nc = bacc.Bacc("TRN2", target_bir_lowering=False, debug=False,
                   num_devices=N_CORES)
    p_d = nc.declare_dram_parameter("p", [128, 4096], BF16, isOutput=False)
    wts_d = nc.declare_dram_parameter("wts", [128, 1024], BF16, isOutput=False)
    aux_d = nc.declare_dram_parameter("aux", [128, AUXTOT], F32, isOutput=False)
    out_d = nc.declare_dram_parameter("out", [2, 32], F32, isOutput=True)

    act_flags = [(((i + 1) * _ACT_SHARE) // _N_GROUPS) > ((i * _ACT_SHARE) // _N_GROUPS)
                 for i in range(_N_GROUPS)]

    with tile.TileContext(nc) as tc:
        with (
            tc.tile_pool(name="persist", bufs=1) as persist,
            tc.tile_pool(name="psum", bufs=2, space="PSUM") as psum,
        ):
            wts_t = persist.tile([128, 1024], BF16)
            nc.sync.dma_start(out=wts_t, in_=wts_d[:, :])

            aux_t = persist.tile([128, AUXTOT], F32)
            nc.gpsimd.dma_start(out=aux_t, in_=aux_d[:, :])

            ptiles = []
            for q in range(8):
                pt_in = persist.tile([128, 512], BF16, tag=f"p{q}")
                eng = nc.sync if q < 4 else nc.gpsimd
                eng.dma_start(
                    out=pt_in, in_=p_d[:, q * 512:(q + 1) * 512])
                ptiles.append(pt_in)

            acc_t = persist.tile([128, 64], F32)
            zeros_t = persist.tile([128, 1], F32)
            nc.vector.memset(zeros_t, 0.0)

            # Main loop: per (k, g4) produce TWO 2-bank psum groups — image
            # 0 (p partitions 0:64, PE row group 0) and image 1 (partitions
            # 64:128, row group 64). Matmuls of the two groups are
            # interleaved so consecutive MMs target different PE row groups:
            # the PE pulls the next LDWEIGHTS ahead and runs both sub-array
            # halves concurrently. bufs=2 per tag -> PE writes iteration i+1
            # while the relu engines (ScalarE for image 0, VectorE for image
            # 1, concurrently) drain iteration i.
            for k in range(8):
                b1_ap = aux_t[:, AUXB1 + k:AUXB1 + k + 1]
                for g in range(4):  # 1024-patch group
                    psA = psum.tile([128, 1024], F32, tag="psA", bufs=2)
                    psB = psum.tile([128, 1024], F32, tag="psB", bufs=2)
                    for j in range(2):
                        t = 2 * g + j
                        nc.tensor.matmul(
                            psA[:, 512 * j:512 * j + 512],
                            lhsT=wts_t[0:64, 128 * k:128 * k + 128],
                            rhs=ptiles[t][0:64, :],
                            start=True, stop=True,
                        )
                        nc.tensor.matmul(
                            psB[:, 512 * j:512 * j + 512],
                            lhsT=wts_t[64:128, 128 * k:128 * k + 128],
                            rhs=ptiles[t][64:128, :],
                            start=True, stop=True,
                        )
                    accA = acc_t[:, 8 * k + g:8 * k + g + 1]
                    if 4 * k + g == 17:
                        # lane rebalance: ScalarE costs ~1372ns/tile vs
                        # VectorE ~1291ns; 31/33 split evens the two lanes.
                        nc.vector.scalar_tensor_tensor(
                            out=psA, in0=psA, scalar=b1_ap,
                            in1=zeros_t.to_broadcast([128, 1024]),
                            op0=mybir.AluOpType.add, op1=mybir.AluOpType.max,
                            accum_out=accA,
                        )
                    else:
                        nc.scalar.activation(
                            psA, psA, mybir.ActivationFunctionType.Relu,
                            bias=b1_ap, scale=1.0, accum_out=accA,
                        )
                    # out = max(psB + b1, 0); accum_out = sum(out)
                    nc.vector.scalar_tensor_tensor(
                        out=psB, in0=psB, scalar=b1_ap,
                        in1=zeros_t.to_broadcast([128, 1024]),
                        op0=mybir.AluOpType.add, op1=mybir.AluOpType.max,
                        accum_out=acc_t[:, 8 * k + 4 + g:8 * k + 4 + g + 1],
                    )

            # s[:, 2k+img] = sum_g acc[:, 8k+4img+g]
            s_t = persist.tile([128, 16], F32)
            nc.vector.tensor_reduce(
                out=s_t,
                in_=acc_t.rearrange("p (kh g) -> p kh g", g=4),
                axis=mybir.AxisListType.X,
                op=mybir.AluOpType.add,
            )

            # out[img, :] = sum_k s[:, 2k+img]^T @ M2e_k  + b2
            ps_f = psum.tile([128, 1024], F32, tag="psA", bufs=2)
            for k in range(8):
                nc.tensor.matmul(
                    ps_f[0:2, 0:32],
                    lhsT=s_t[:, 2 * k:2 * k + 2],
                    rhs=aux_t[:, AUXM2 + 32 * k:AUXM2 + 32 * k + 32],
                    start=(k == 0), stop=(k == 7),
                )
            out_sb = persist.tile([2, 32], F32)
            nc.vector.tensor_tensor(
                out=out_sb, in0=ps_f[0:2, 0:32], in1=aux_t[0:2, AUXB2:AUXB2 + 32],
                op=mybir.AluOpType.add,
            )
            nc.sync.dma_start(out=out_d[:, :], in_=out_sb)

    nc.compile()
    return nc


_NC_CACHE = None
TRACE = False
_last_result = None
_last_profile_dir = None


def _get_nc():
    global _NC_CACHE
    if _NC_CACHE is None:
        _NC_CACHE = _build_nc()
    return _NC_CACHE


def kernel(x, w1, b1, w2, b2):
    global _last_result
    x = np.ascontiguousarray(np.asarray(x, dtype=np.float32))
    Wt, b1c, M2c, b2t = _build_weights(
        np.asarray(w1, np.float32), np.asarray(b1, np.float32),
        np.asarray(w2, np.float32), np.asarray(b2, np.float32))

    wts = np.empty((128, 1024), dtype=NP_BF16)
    wts[0:64] = Wt.astype(NP_BF16)
    wts[64:128] = wts[0:64]
    aux = np.empty((128, AUXTOT), dtype=np.float32)
    aux[:, AUXB1:AUXB1 + 8] = b1c
    aux[:, AUXM2:AUXM2 + 256] = M2c
    aux[:, AUXB2:AUXB2 + 32] = b2t

    # patches: x [16,1,512,512] -> [b, pixel(r,c), patch(i,j)] = [16, 64, 4096]
    p_all = (x.reshape(16, 64, 8, 64, 8).transpose(0, 2, 4, 1, 3)
             .reshape(16, 64, 4096).astype(NP_BF16))

    in_maps = []
    for c in range(N_CORES):
        pc = np.empty((128, 4096), dtype=NP_BF16)
        pc[0:64] = p_all[2 * c]
        pc[64:128] = p_all[2 * c + 1]
        in_maps.append({"p": pc, "wts": wts, "aux": aux})

    nc = _get_nc()
    if TRACE:
        # Local profiling path: NTFF via the axon hook, processed locally
        # (run_bass_kernel_spmd's trace arm hangs on artifact upload here).
        import tempfile
        from concourse import bass2jax
        from antenv.axon_hooks import get_axon_ntff_profile_hook

        global _last_profile_dir
        hook = get_axon_ntff_profile_hook()
        tmpdir = tempfile.mkdtemp(prefix="dctcnn_prof_")
        with hook(tmpdir, [0]):
            results = bass2jax.run_bass_via_pjrt(nc, in_maps, n_cores=N_CORES)
        _last_profile_dir = tmpdir
        out = np.concatenate([results[c]["out"] for c in range(N_CORES)], axis=0)
        return out.astype(np.float32)
    res = run_bass_kernel_spmd(nc, in_maps, list(range(N_CORES)))
    _last_result = res
    out = np.concatenate([res.results[c]["out"] for c in range(N_CORES)], axis=0)
    return out.astype(np.float32)
